# revision 10
# baseline (speedup 1.0000x reference)
"""Trainium2 Bass kernel for the quantum ConvLSTM reference.

Math reduction (validated to ~3.6e-3 vs the jax reference, budget 2e-2):
  * quantum_conv(patch) == T16[b], evaluated as a multilinear polynomial
    in the 4 threshold bits (Pool engine, chunked, overlapped).
  * qlayer(x, p) == products of z_w = cos(x_w + p_w); the CNOT ring makes
    <Z> factorize into products of cosines.
  * The LSTM scan is a 512-step serial recurrence evaluated with 12 DVE
    ops per step:
      2x fused multiply-cumsum (theta_h per batch group)
      1  page-end difference (theta_h extraction)
      1  add+wrap into [-pi, pi]  (pre includes beta + rx + pi/2)
      1  deg-7 odd sin polynomial (single custom op, C3 via in1)
      3  gate products (pair ops via strided APs)
      1  deg-7 odd tanh polynomial (single custom op)
      1  (1+t)*x pair op (0.5 folded)   -> f*c and i*g halves
      1  add                            -> c update
      1  fused deg-5 half-tanh * (1+o)  -> h update (poly lead folded
                                           into wh / w_out packing)
    No scalar-engine (ACT) op in the loop; conv/pre/output phases run on
    the Pool engine in t-chunks, pipelined around the DVE loop.

Sharding: pure data parallel over batch (2048 -> 8 cores x 256 rows).
Each core: 128 partitions x 2 column-groups.
"""

import sys

import numpy as np

sys.path.insert(0, "/opt/trn_rl_repo")

N_CORES = 8
BATCH = 2048
SEQ = 512
BPC = BATCH // N_CORES          # 256 batch rows per core
GRP = BPC // 128                # 2 column groups of 128 partitions
PI = float(np.pi)

# t-chunks: small first chunk so the DVE loop starts early while the Pool
# engine streams the remaining conv/pre chunks.
CHUNKS = [32, 96, 128, 128, 128]
CSTARTS = [0, 32, 128, 256, 384]

# Gate order in the on-chip layout (u third so [c,f,i,u,o] slot strides
# work out); wire (qlayer component) order [q1,q0,q2,q3]; theta stream
# wire order [3,2,1,0].
GATE_ORDER = ["f", "i", "u", "o"]
WIRE_OF_K = [1, 0, 2, 3]        # stored h component k -> wire
WIRE_OF_POS = [3, 2, 1, 0]      # theta stream position -> wire

# ---------------------------------------------------------------------------
# Host-side constants: T16 lookup table + multilinear coefficients.
# ---------------------------------------------------------------------------
_RY_ANGLES = np.random.RandomState(0).uniform(0.0, 2.0 * np.pi, size=(2, 4)).astype(np.float32)


def _build_t16() -> np.ndarray:
    s = np.zeros((16, 2, 2, 2, 2), np.complex64)
    for b in range(16):
        bits = [(b >> 3) & 1, (b >> 2) & 1, (b >> 1) & 1, b & 1]
        s[(b, *bits)] = 1.0

    def ry(state, th, w):
        a0 = np.take(state, 0, axis=1 + w)
        a1 = np.take(state, 1, axis=1 + w)
        c = np.complex64(np.cos(np.float32(th) / 2))
        sn = np.complex64(np.sin(np.float32(th) / 2))
        return np.stack([c * a0 - sn * a1, sn * a0 + c * a1], axis=1 + w)

    def cnot(state, ctl, tgt):
        s0 = np.take(state, 0, axis=1 + ctl)
        s1 = np.take(state, 1, axis=1 + ctl)
        t_ax = 1 + tgt if tgt < ctl else tgt
        s1 = np.flip(s1, axis=t_ax)
        return np.stack([s0, s1], axis=1 + ctl)

    for layer in range(2):
        for w in range(4):
            s = ry(s, _RY_ANGLES[layer, w], w)
        for w in range(3):
            s = cnot(s, w, w + 1)
    probs = np.abs(s) ** 2
    cols = []
    for w in range(4):
        other = tuple(a for a in range(1, 5) if a != 1 + w)
        cols.append(probs.sum(axis=other)[:, 1])
    return np.stack(cols, axis=1).mean(axis=1)  # (16,)


def _multilinear_coeffs(t16: np.ndarray) -> np.ndarray:
    """C[4][4] with T16[b] = sum_jk C[j,k]*u_j*v_k, u=[1,b0,b1,b0b1], v=[1,b2,b3,b2b3]."""
    m = np.zeros((16, 16))
    for b in range(16):
        b0, b1, b2, b3 = (b >> 3) & 1, (b >> 2) & 1, (b >> 1) & 1, b & 1
        u = [1, b0, b1, b0 * b1]
        v = [1, b2, b3, b2 * b3]
        for j in range(4):
            for k in range(4):
                m[b, j * 4 + k] = u[j] * v[k]
    return np.linalg.solve(m, t16.astype(np.float64)).reshape(4, 4)


_T16 = _build_t16()
_CML = _multilinear_coeffs(_T16)


def _fit_odd(f, hi, ncoef):
    """Near-minimax odd fit f(x) ~ x*p(x^2) on [-hi, hi]; returns p coeffs
    c[0..ncoef-1] (ascending powers of x^2)."""
    n = 4000
    k = np.arange(n)
    x = np.cos(np.pi * (k + 0.5) / n) * hi
    y = x * x
    a = np.stack([x * y ** j for j in range(ncoef)], axis=1)
    tgt = f(x)
    c, *_ = np.linalg.lstsq(a, tgt, rcond=None)
    for _ in range(60):
        r = a @ c - tgt
        w = (np.abs(r) + 1e-12) ** 0.5
        c, *_ = np.linalg.lstsq(a * w[:, None], tgt * w, rcond=None)
    return c


_SIN7 = _fit_odd(np.sin, np.pi, 4)          # deg-7 odd sin on [-pi, pi]
_TANH7 = _fit_odd(np.tanh, 1.0, 4)          # deg-7 odd tanh on [-1, 1]
_HT5 = _fit_odd(lambda v: 0.5 * np.tanh(v), 1.58, 3)  # deg-5 odd 0.5*tanh
# h update op computes c*(s^2 + d1*s + d0)*(1+o); lead coeff folded into
# wh / w_out packing.
_HT_FOLD = float(_HT5[2])
_HT_D1 = float(_HT5[1] / _HT5[2])
_HT_D0 = float(_HT5[0] / _HT5[2])

_CACHE = {}


def _register_custom_ops():
    """Register fused DVE ops (idempotent). Shas are pinned by bootstrap:
    compile once with an empty pin, parse the actual sha from the error."""
    import re
    import concourse.dve_ops as dve_ops_mod
    from concourse.dve_ops import OPS, DveOp
    from concourse.dve_spec import (
        Spec, Src0, Src1, C0, C1, C2, C3, Zero, One, scan, AluOp,
        _spill_c3_to_src1,
    )

    have = {o.name for o in OPS}

    def make(name, spec):
        if name in have:
            return next(o for o in OPS if o.name == name)
        probe = DveOp(name, spec, subdim=False, uops_sha={})
        OPS.append(probe)
        dve_ops_mod._SUB_OPCODE_FOR_NAME[name] = (
            dve_ops_mod._CUSTOM_DVE_ROW_BASE + len(OPS) - 1)
        shas = {}
        for ver in ("v3", "v4"):
            try:
                probe.compile(ver)
            except ValueError as e:
                mm = re.search(r"(\b[0-9a-f]{16})\b", str(e))
                if mm is None:
                    raise
                shas[ver] = mm.group(1)
        OPS.remove(probe)
        op = DveOp(name, spec, subdim=False, uops_sha=shas)
        OPS.append(op)
        return op

    # out = cumsum(in0 * in1) along the free stream
    def _ref_mulscan(in0, in1, c0, c1, c2):
        p = (in0.astype(np.float32) * in1).reshape(in0.shape[0], -1)
        return np.cumsum(p, axis=1, dtype=np.float32).reshape(in0.shape)

    mulscan = make("MULSCAN_ANT", Spec(
        body=scan(AluOp.ADD, Src0 * Src1),
        reference=_ref_mulscan))

    # out = wrap(in0 + in1 + c0) into [-c1, c1] by one period imm2
    y = (Src0 + Src1) + C0
    addwrap2 = make("ADDWRAP2_ANT", Spec(
        body=y + C2 * ((y < (Zero - C1)) - (C1 < y)),
        reference=lambda in0, in1, c0, c1, c2: (
            lambda yy: (yy + c2 * ((yy < -c1).astype(np.float32)
                                   - (c1 < yy).astype(np.float32))
                        ).astype(np.float32))(
            (in0.astype(np.float32) + in1 + c0).astype(np.float32))))

    # out = (((c0*s + c1)*s + c2)*s + c3)*in0, s = in0^2 — deg-7 odd poly.
    # c3 rides in1 (a [P,1] column) via the C3 spill.
    s = Src0 * Src0
    odp7_body = _spill_c3_to_src1((((C0 * s + C1) * s + C2) * s + C3) * Src0)

    def _ref_odp7(in0, in1, c0, c1, c2):
        x = in0.astype(np.float32)
        ss = x * x
        c3 = np.asarray(in1, np.float32).reshape(in0.shape[0], *([1] * (in0.ndim - 1)))
        return ((((c0 * ss + c1) * ss + c2) * ss + c3) * x).astype(np.float32)

    odp7 = make("ODP7_ANT", Spec(body=odp7_body, reference=_ref_odp7))

    # out = (in0 + 1) * in1 * c0  (pairwise gate*state products)
    p1p = make("P1PH_ANT", Spec(
        body=(Src0 + One) * Src1 * C0,
        reference=lambda in0, in1, c0, c1, c2: (
            (in0.astype(np.float32) + 1.0) * in1 * c0).astype(np.float32)))

    # out = ((s + c0)*s + c1)*in0*(in1 + 1), s = in0^2 — fused h update
    sh = Src0 * Src0
    hnew = make("HNEW5_ANT", Spec(
        body=((sh + C0) * sh + C1) * Src0 * (Src1 + One),
        reference=lambda in0, in1, c0, c1, c2: (
            lambda x, ss: (((ss + c0) * ss + c1) * x
                           * (in1.astype(np.float32) + 1.0)).astype(np.float32))(
            in0.astype(np.float32), in0.astype(np.float32) ** 2)))

    return mulscan, addwrap2, odp7, p1p, hnew


def _build_program():
    """Build + compile the (weights-independent) single-core SPMD Bass program."""
    import concourse.mybir as mybir
    import concourse.tile as tile
    from concourse import bacc

    F32 = mybir.dt.float32
    OP = mybir.AluOpType

    MULSCAN, ADDWRAP2, ODP7, P1PH, HNEW5 = _register_custom_ops()
    sc = [float(v) for v in _SIN7]
    tc_ = [float(v) for v in _TANH7]

    nc = bacc.Bacc(None, target_bir_lowering=False)

    x_d = nc.dram_tensor("xs", [BPC, SEQ * 4], F32, kind="ExternalInput")
    wh_d = nc.dram_tensor("wh", [128, 128], F32, kind="ExternalInput")
    cp_d = nc.dram_tensor("cp", [128, 39], F32, kind="ExternalInput")
    y_d = nc.dram_tensor("y", [BPC, SEQ], F32, kind="ExternalOutput")

    NCH = len(CHUNKS)

    with tile.TileContext(nc) as tcx:
        with (
            tcx.tile_pool(name="big", bufs=1) as big,
            tcx.tile_pool(name="scr", bufs=2) as scr,
        ):
            whsb = big.tile([128, 128], F32, tag="WH")
            nc.sync.dma_start(whsb[:], wh_d[:])
            cpsb = big.tile([128, 39], F32, tag="CP")
            nc.sync.dma_start(cpsb[:], cp_d[:])

            # ---------- per-chunk input DMA + Pool phases 1/1b ----------
            pre_t, hs_t, y_t = [], [], []
            P = nc.gpsimd
            for ci, (t0, tcn) in enumerate(zip(CSTARTS, CHUNKS)):
                xsb = scr.tile([128, tcn * GRP * 4], F32, tag="X", name=f"X{ci}")
                nc.sync.dma_start(
                    xsb[:].rearrange("p (g t k) -> p g t k", g=GRP, k=4),
                    x_d[:, t0 * 4:(t0 + tcn) * 4]
                        .rearrange("(g p) (t k) -> p g t k", p=128, k=4),
                )
                bits = scr.tile([128, tcn * GRP * 4], F32, tag="B", name=f"Bc{ci}")
                P.tensor_scalar(out=bits[:], in0=xsb[:], scalar1=127.0,
                                scalar2=None, op0=OP.is_gt)
                bv = bits[:].rearrange("p (g t k) -> p g t k", g=GRP, k=4)
                bk = [bv[:, :, :, k] for k in range(4)]       # each (128, g, t)

                def ctile(tag):
                    return scr.tile([128, tcn * GRP], F32, tag=tag,
                                    name=f"{tag}{ci}")
                q23 = ctile("q23")
                mm = ctile("mm")
                gt = lambda tl: tl[:].rearrange("p (g t) -> p g t", g=GRP)
                P.tensor_tensor(out=gt(q23), in0=bk[2], in1=bk[3], op=OP.mult)
                rs = []
                for j in range(4):
                    r = ctile(f"r{j}_")
                    P.tensor_scalar(out=gt(r), in0=bk[2],
                                    scalar1=float(_CML[j, 1]),
                                    scalar2=float(_CML[j, 0]),
                                    op0=OP.mult, op1=OP.add)
                    P.tensor_scalar(out=gt(mm), in0=bk[3],
                                    scalar1=float(_CML[j, 2]),
                                    scalar2=None, op0=OP.mult)
                    P.tensor_tensor(out=gt(r), in0=gt(r), in1=gt(mm), op=OP.add)
                    P.tensor_scalar(out=gt(mm), in0=gt(q23),
                                    scalar1=float(_CML[j, 3]),
                                    scalar2=None, op0=OP.mult)
                    P.tensor_tensor(out=gt(r), in0=gt(r), in1=gt(mm), op=OP.add)
                    rs.append(r)
                q01 = ctile("q01")
                P.tensor_tensor(out=gt(q01), in0=bk[0], in1=bk[1], op=OP.mult)
                m = ctile("m_")
                vcv = ctile("V")
                P.tensor_tensor(out=gt(m), in0=bk[0], in1=gt(rs[1]), op=OP.mult)
                P.tensor_tensor(out=gt(vcv), in0=gt(rs[0]), in1=gt(m), op=OP.add)
                P.tensor_tensor(out=gt(m), in0=bk[1], in1=gt(rs[2]), op=OP.mult)
                P.tensor_tensor(out=gt(vcv), in0=gt(vcv), in1=gt(m), op=OP.add)
                P.tensor_tensor(out=gt(m), in0=gt(q01), in1=gt(rs[3]), op=OP.mult)
                P.tensor_tensor(out=gt(vcv), in0=gt(vcv), in1=gt(m), op=OP.add)

                # pre[t,g,a,pos] = wx*conv + (beta + rx + pi/2)
                pre = big.tile([128, tcn * GRP * 16], F32, tag=f"PRE{ci}", name=f"PRE{ci}")
                pv = pre[:].rearrange("p (t g m) -> p t g m", g=GRP, m=16)
                cvb = (vcv[:].rearrange("p (g t) -> p g t", g=GRP)
                       .transpose([0, 2, 1])
                       .unsqueeze(3).broadcast_to((128, tcn, GRP, 16)))
                wxb = (cpsb[:, 0:16].unsqueeze(1).unsqueeze(1)
                       .broadcast_to((128, tcn, GRP, 16)))
                btb = (cpsb[:, 16:32].unsqueeze(1).unsqueeze(1)
                       .broadcast_to((128, tcn, GRP, 16)))
                P.tensor_tensor(out=pv, in0=cvb, in1=wxb, op=OP.mult)
                P.tensor_tensor(out=pv, in0=pv, in1=btb, op=OP.add)
                pre_t.append(pre)

                hs_t.append(big.tile([128, tcn * GRP * 4], F32, tag=f"HS{ci}", name=f"HS{ci}"))
                y_t.append(big.tile([128, tcn * GRP], F32, tag=f"Y{ci}", name=f"Yc{ci}"))

            # ---------------- DVE state tiles + init ----------------
            V = nc.vector
            h0t = big.tile([128, GRP * 4], F32, tag="H0")
            V.memset(h0t[:], 0.0)
            sg = big.tile([128, GRP * 68], F32, tag="SG")
            V.memset(sg[:], 0.0)
            z8 = big.tile([128, GRP * 4 * 8], F32, tag="Z8")
            z8g = z8[:].rearrange("p (ga c) -> p ga c", c=8)
            V.memset(z8g[:, :, 4:5], 0.5)        # cg for sigmoid gates
            z84 = z8[:].rearrange("p (g a c) -> p g a c", g=GRP, c=8)
            V.memset(z84[:, :, 2, 4:5], 1.0)     # gate u (index 2): cg = 1
            t5 = big.tile([128, GRP * 24], F32, tag="T5")   # slots [c,f,i,u,o,pad]
            t5g = t5[:].rearrange("p (g c) -> p g c", g=GRP)
            V.memset(t5g[:, :, 0:4], 0.0)        # c state = 0
            qt = big.tile([128, GRP * 16], F32, tag="QT")
            uv = big.tile([128, GRP * 8], F32, tag="UV")
            th = big.tile([128, GRP * 16], F32, tag="TH")
            wr = big.tile([128, GRP * 16], F32, tag="WR")

            sin_c3 = cpsb[:, 37:38]
            tanh_c3 = cpsb[:, 38:39]

            # z8 per-gate views for the product ops
            zv_m_in0 = z8g[:, :, 1:3]                                  # (z2, z1)
            zv_m_in1 = z8[:].rearrange("p (ga r c) -> p ga r c", r=2, c=4)[:, :, :, 0]  # (z3, cg)
            zv_m_out = z8g[:, :, 5:7]                                  # (b23, z1')
            zv_p1_in1 = z8[:].rearrange("p (ga r c) -> p ga r c", r=4, c=2)[:, :, 1:3, 1]  # (z0, b23)
            zv_p1_in0 = z8g[:, :, 6:7].broadcast_to((128, GRP * 4, 2))  # z1' x2
            zv_p2_in1 = z8[:].rearrange("p (ga r c) -> p ga r c", r=2, c=4)[:, :, :, 1]  # (z2, b23)
            qv = qt[:].rearrange("p (ga w) -> p ga w", w=4)
            zv_p2_in0 = qv[:, :, 0:1].broadcast_to((128, GRP * 4, 2))   # q1 x2
            # t5 views
            t_fused_in0 = t5g[:, :, 4:12]                               # (t_f, t_i)
            t_fused_in1 = t5[:].rearrange("p (a b w) -> p a b w", a=4, w=4)[:, :, 0, :]  # (c,u)x(g)
            t_tanh_out = t5g[:, :, 4:20]                                # slots f,i,u,o
            t_c = t5g[:, :, 0:4]
            t_o = t5g[:, :, 16:20]
            uvv = uv[:].rearrange("p (g j w) -> p g j w", g=GRP, w=4)
            sgv = sg[:].rearrange("p (g c) -> p g c", g=GRP)
            ends = sgv[:, :, 4:68].rearrange("p g (m k) -> p g m k", k=4)[:, :, :, 0]
            starts = sgv[:, :, 0:64].rearrange("p g (m k) -> p g m k", k=4)[:, :, :, 0]

            def h_ap(u):
                """(p, GRP*4) view of h at global position u."""
                if u == 0:
                    return h0t[:]
                ci = next(i for i in range(NCH)
                          if CSTARTS[i] < u <= CSTARTS[i] + CHUNKS[i])
                off = (u - CSTARTS[ci] - 1) * GRP * 4
                return hs_t[ci][:, off:off + GRP * 4]

            # ---------------- the 512-step scan ----------------
            for t in range(SEQ):
                ci = next(i for i in range(NCH)
                          if CSTARTS[i] <= t < CSTARTS[i] + CHUNKS[i])
                hv = h_ap(t).rearrange("p (g k) -> p g k", g=GRP)
                for g in range(GRP):
                    hb = hv[:, g].unsqueeze(1).broadcast_to((128, 16, 4))
                    V._custom_dve(
                        MULSCAN,
                        out=sgv[:, g, 1:65].rearrange("p (r k) -> p r k", k=4),
                        in0=hb,
                        in1=whsb[:, g * 64:(g + 1) * 64]
                            .rearrange("p (r k) -> p r k", k=4))
                V.tensor_tensor(out=th[:].rearrange("p (g m) -> p g m", g=GRP),
                                in0=ends, in1=starts, op=OP.subtract)
                off = (t - CSTARTS[ci]) * GRP * 16
                V._custom_dve(ADDWRAP2, out=wr[:], in0=th[:],
                              in1=pre_t[ci][:, off:off + GRP * 16],
                              s0=0.0, s1=PI, imm2=2 * PI)
                V._custom_dve(ODP7, out=z8g[:, :, 0:4], in0=wr[:], in1=sin_c3,
                              s0=sc[3], s1=sc[2], imm2=sc[1])
                V.tensor_tensor(out=zv_m_out, in0=zv_m_in0, in1=zv_m_in1,
                                op=OP.mult)
                V.tensor_tensor(out=qv[:, :, 0:2], in0=zv_p1_in0,
                                in1=zv_p1_in1, op=OP.mult)
                V.tensor_tensor(out=qv[:, :, 2:4], in0=zv_p2_in0,
                                in1=zv_p2_in1, op=OP.mult)
                V._custom_dve(ODP7, out=t_tanh_out, in0=qt[:], in1=tanh_c3,
                              s0=tc_[3], s1=tc_[2], imm2=tc_[1])
                V._custom_dve(P1PH, out=uv[:], in0=t_fused_in0,
                              in1=t_fused_in1, s0=0.5)
                V.tensor_tensor(out=t_c, in0=uvv[:, :, 0], in1=uvv[:, :, 1],
                                op=OP.add)
                V._custom_dve(HNEW5, out=h_ap(t + 1), in0=t_c, in1=t_o,
                              s0=_HT_D1, s1=_HT_D0)

            # ---------------- phase 3 (Pool, per chunk) + DMA out ------
            wob = (cpsb[:, 32:36].unsqueeze(1).unsqueeze(1)
                   .broadcast_to((128, 1, GRP, 4)))
            for ci, (t0, tcn) in enumerate(zip(CSTARTS, CHUNKS)):
                hsv = hs_t[ci][:].rearrange("p (t g w) -> p t g w", g=GRP, w=4)
                yt = scr.tile([128, tcn * GRP * 4], F32, tag="YT", name=f"YT{ci}")
                ytv = yt[:].rearrange("p (t g w) -> p t g w", g=GRP, w=4)
                P.tensor_tensor(out=ytv, in0=hsv,
                                in1=wob.broadcast_to((128, tcn, GRP, 4)),
                                op=OP.mult)
                ya = scr.tile([128, tcn * GRP], F32, tag="YA", name=f"YA{ci}")
                yav = ya[:].rearrange("p (t g) -> p t g", g=GRP)
                ybv = (y_t[ci][:].rearrange("p (g t) -> p g t", g=GRP)
                       .transpose([0, 2, 1]))
                P.tensor_tensor(out=yav, in0=ytv[:, :, :, 0],
                                in1=ytv[:, :, :, 1], op=OP.add)
                P.tensor_tensor(out=ybv, in0=ytv[:, :, :, 2],
                                in1=ytv[:, :, :, 3], op=OP.add)
                P.tensor_tensor(out=ybv, in0=ybv, in1=yav, op=OP.add)
                P.tensor_scalar(out=y_t[ci][:], in0=y_t[ci][:],
                                scalar1=cpsb[:, 36:37], scalar2=None,
                                op0=OP.add)
                nc.sync.dma_start(
                    y_d[:, t0:t0 + tcn].rearrange("(g p) t -> p g t", p=128),
                    y_t[ci][:].rearrange("p (g t) -> p g t", g=GRP),
                )

    nc.compile()
    return nc


def _pack_consts(W_f, b_f, W_i, b_i, W_u, b_u, W_o, b_o,
                 rx_f, rx_i, rx_u, rx_o, W_out, b_out):
    """wh[128,128] and cp[128,39] constant tiles (replicated rows)."""
    Wd = {"f": W_f, "i": W_i, "u": W_u, "o": W_o}
    bd = {"f": b_f, "i": b_i, "u": b_u, "o": b_o}
    rd = {"f": rx_f, "i": rx_i, "u": rx_u, "o": rx_o}

    # wh[g, a, pos, k] = fold * W_a[wire_of_pos[pos], 1 + wire_of_k[k]]
    whrow = np.zeros((GRP, 4, 4, 4), np.float32)
    for a, gn in enumerate(GATE_ORDER):
        Wg = np.asarray(Wd[gn], np.float32)
        for pos in range(4):
            for k in range(4):
                whrow[:, a, pos, k] = (
                    _HT_FOLD * Wg[WIRE_OF_POS[pos], 1 + WIRE_OF_K[k]])
    wh = np.tile(whrow.reshape(1, 128), (128, 1)).astype(np.float32)

    cprow = np.zeros(39, np.float32)
    for a, gn in enumerate(GATE_ORDER):
        Wg = np.asarray(Wd[gn], np.float32)
        bg = np.asarray(bd[gn], np.float32) + np.asarray(rd[gn], np.float32)
        for pos in range(4):
            w = WIRE_OF_POS[pos]
            cprow[a * 4 + pos] = Wg[w, 0]
            cprow[16 + a * 4 + pos] = bg[w] + PI / 2
    wo = np.asarray(W_out, np.float32)[0]
    for k in range(4):
        cprow[32 + k] = _HT_FOLD * wo[WIRE_OF_K[k]]
    cprow[36] = float(np.asarray(b_out, np.float32)[0])
    cprow[37] = float(_SIN7[0])
    cprow[38] = float(_TANH7[0])
    cp = np.tile(cprow[None], (128, 1)).astype(np.float32)

    # range check for the single add_range_wrap before the sin poly
    wx = np.abs(cprow[0:16])
    beta = np.abs(cprow[16:32] )
    whabs = np.abs(whrow[0]).reshape(16, 4).sum(axis=1) / abs(_HT_FOLD)
    bound = (wx + beta + whabs).max()
    assert bound < 3 * PI - 0.2, f"theta range {bound} too large for single wrap"
    return wh, cp


def kernel(**inputs):
    from concourse.bass_utils import run_bass_kernel_spmd

    x = np.ascontiguousarray(np.asarray(inputs["x"], np.float32)).reshape(BATCH, SEQ, 4)
    wh, cp = _pack_consts(**{k: v for k, v in inputs.items() if k != "x"})

    if "nc" not in _CACHE:
        _CACHE["nc"] = _build_program()
    nc = _CACHE["nc"]

    in_maps = []
    for cid in range(N_CORES):
        xs = np.ascontiguousarray(
            x[cid * BPC:(cid + 1) * BPC].reshape(BPC, SEQ * 4))
        in_maps.append({"xs": xs, "wh": wh, "cp": cp})

    res = run_bass_kernel_spmd(nc, in_maps, core_ids=list(range(N_CORES)))
    ys = [res.results[cid]["y"] for cid in range(N_CORES)]  # each (BPC, SEQ)
    full = np.concatenate(ys, axis=0)                       # (BATCH, SEQ)
    return np.ascontiguousarray(full.T)[:, :, None].astype(np.float32)
